# revision 29
# baseline (speedup 1.0000x reference)
"""Trainium2 Bass kernel for nn_Expert_13082470383822.

y = silu(depthwise_causal_conv1d(x, conv_w, K=4) + conv_b);  out = y @ W_proj.T + b_proj
x [4, 4096, 2048] fp32. Data-parallel over the 16384 (batch*seq) tokens across
8 NeuronCores (2048 tokens/core + 3-token halo).

Per-core: channels D on SBUF partitions. The whole data path runs in bf16 (host
casts x and W_proj; end-to-end error ~5e-3 vs the 2e-2 gate), which halves the
W stream (the fp32 baseline stalled the PE ~17us midway waiting on W tiles) and
the x strips. Conv strips are 256 tokens for the first two (short startup
chain), 512 after (amortizes the ~200-cycle fixed cost of each DVE/ACT op so
the conv no longer outpaces the PE's per-strip span). Per channel-tile: tap 0
on ACT (copy with per-partition scale), taps 1-3 as DVE scalar_tensor_tensor
chains (stt has no 2x uop - it runs 1x regardless of dtype), SiLU+conv_b on ACT
writing bf16 y tiles. Projection on the PE in bf16 (1 cycle/row, FWL weight
loads) accumulating fp32 in PSUM; b_proj added on the DVE drain. Output
streams out as [tokens, 2048] fp32 rows so the host gather is concatenation.
"""

import sys

if "/opt/trn_rl_repo" not in sys.path:
    sys.path.insert(0, "/opt/trn_rl_repo")

import numpy as np
import ml_dtypes

BF16 = ml_dtypes.bfloat16

B, S, D, KW = 4, 4096, 2048, 4
NCORES = 8
T = (B * S) // NCORES  # tokens per core = 2048
KT = D // 128  # 16 channel tiles
ECH = D // 512  # 4 e-chunks
MS = 128  # matmul strip width (tokens)
SW = [256] * 8  # conv strip widths (sum = T)
assert sum(SW) == T

_BUILT = {}


def _build_program():
    if "nc" in _BUILT:
        return _BUILT["nc"]

    import concourse.tile as tile
    from concourse import bacc, mybir

    dt = mybir.dt
    AF = mybir.ActivationFunctionType
    ALU = mybir.AluOpType

    nc = bacc.Bacc("TRN2", target_bir_lowering=False, debug=False)
    # pre-tiled x, one param per strip width, one [128, 16*(w+3)] block per
    # strip: 8.3KB contiguous per partition per DMA - small-descriptor
    # penalty avoided (4KB/partition descriptors only reach ~160GB/s)
    xs_p = {}
    for w in sorted(set(SW)):
        nblk = sum(1 for v in SW if v == w)
        xs_p[w] = nc.declare_dram_parameter(
            f"xs_{w}", [nblk, 128, KT * (w + 3)], dt.bfloat16, isOutput=False
        )
    # W_proj^T packed as 8 channel-tile PAIRS: each DMA moves [128, 2, 2048]
    # = 8KB per partition, twice the descriptor size of a single tile
    wt = nc.declare_dram_parameter(
        "wt", [KT // 2, 128, 2 * D], dt.bfloat16, isOutput=False
    )
    cw = nc.declare_dram_parameter("cw", [128, KT * KW], dt.float32, isOutput=False)
    cb = nc.declare_dram_parameter("cb", [128, KT], dt.float32, isOutput=False)
    bp = nc.declare_dram_parameter("bp", [1, D], dt.float32, isOutput=False)
    out = nc.declare_dram_parameter("out", [T, D], dt.float32, isOutput=True)

    with tile.TileContext(nc) as tc:
        with (
            tc.tile_pool(name="consts", bufs=1) as cpool,
            tc.tile_pool(name="wpool", bufs=1) as wpool,
            tc.tile_pool(name="xpool", bufs=4) as xpool,
            tc.tile_pool(name="ypool", bufs=3) as ypool,
            tc.tile_pool(name="apool", bufs=4) as apool,
            tc.tile_pool(name="opool", bufs=8) as opool,
            tc.tile_pool(name="pspool", bufs=8, space="PSUM") as pspool,
        ):
            xq = []  # (strip) -> list of x tiles (1 block or 2 halves)
            blk_i = {w: 0 for w in xs_p}

            def load_strip(c):
                # strip 0 rides the fast HWDGE sync queue ahead of W so the
                # conv chain starts ~2us earlier; the rest stream on the
                # gpsimd SWDGE queue and never sit behind W
                eng = nc.sync if c < 1 else nc.gpsimd
                w = SW[c]
                xt = xpool.tile([128, KT, w + 3], dt.bfloat16, name="xs", tag="xs")
                sv = xs_p[w][blk_i[w], :, :].rearrange("p (j t) -> p j t", j=KT)
                eng.dma_start(out=xt[:, :, :], in_=sv)
                blk_i[w] += 1
                xq.append(xt)

            dum = cpool.tile([1, 1], dt.float32, name="dum")
            nc.gpsimd.memset(dum[:, :], 0.0)
            # conv scale/bias ride the gpsimd queue head (tiny, land ~8.5us)
            # so the sync queue can lead with the x strips the conv chain
            # needs first, with W right behind
            cw_sb = cpool.tile([128, KT * KW], dt.float32, name="cw_sb")
            nc.gpsimd.dma_start(out=cw_sb[:, :], in_=cw[:, :])
            cb_sb = cpool.tile([128, KT], dt.float32, name="cb_sb")
            nc.gpsimd.dma_start(out=cb_sb[:, :], in_=cb[:, :])
            load_strip(0)
            load_strip(1)
            # warm the ACT function table before any real ACT work
            nc.scalar.activation(dum[:, :], dum[:, :], AF.Silu, bias=0.0)

            # W pairs stream on the SP HWDGE queue behind the strip-0 x
            w_sb = []
            for q in range(KT // 2):
                wq = wpool.tile([128, 2, D], dt.bfloat16, name=f"w{q}")
                nc.sync.dma_start(
                    out=wq[:, :, :],
                    in_=wt[q, :, :].rearrange("p (h c) -> p h c", h=2),
                )
                w_sb.append(wq[:, 0, :])
                w_sb.append(wq[:, 1, :])

            # bias broadcast rides the SP queue behind W: needed only by the
            # first drain, and it stays out of the x-strip FIFO
            bb_sb = cpool.tile([128, D], dt.float32, name="bb_sb")
            nc.sync.dma_start(out=bb_sb[:, :], in_=bp[:, :].broadcast_to([128, D]))

            s = 0  # running matmul-strip index
            for c in range(len(SW)):
                w = SW[c]
                if c + 2 < len(SW):
                    load_strip(c + 2)
                xh = xq[c]

                yt = ypool.tile([128, KT, w], dt.bfloat16, name="ys", tag="ys")

                for j in range(KT):
                    xs, jj = xh, j
                    acc = apool.tile([128, w], dt.bfloat16, name="acc", tag="acc")
                    # tap 0 on ACT: acc = w0 * x0
                    nc.scalar.activation(
                        acc[:, :],
                        xs[:, jj, 0:w],
                        AF.Copy,
                        bias=0.0,
                        scale=cw_sb[:, j * KW : j * KW + 1],
                    )  # tap 0
                    # taps 1-3 on DVE
                    for k in range(1, KW):
                        nc.vector.scalar_tensor_tensor(
                            acc[:, :],
                            xs[:, jj, k : k + w],
                            cw_sb[:, j * KW + k : j * KW + k + 1],
                            acc[:, :],
                            ALU.mult,
                            ALU.add,
                        )
                    # SiLU + conv bias on ACT, full strip width, bf16 out
                    nc.scalar.activation(
                        yt[:, j, :],
                        acc[:, :],
                        AF.Silu,
                        bias=cb_sb[:, j : j + 1],
                    )

                if c == 0:
                    # strip 0: open both matmul strips' PSUM groups at once
                    # (8 banks) and interleave them j-wise for j0-7 - the PE
                    # then wants a channel tile only every ~1.7us while the
                    # conv chain delivers every ~1.2us, so the stream starts
                    # dense instead of conv-paced. j8-15 finish m0 solo (the
                    # conv has banked enough lead by then) so m0's banks
                    # drain while m1 finishes, avoiding a strip-boundary
                    # bubble when strip 1 reuses them.
                    nms = w // MS
                    psall = [
                        [
                            pspool.tile([128, 512], dt.float32, name="ps", tag="ps")
                            for _ in range(ECH)
                        ]
                        for _ in range(nms)
                    ]

                    def mmj(m, j):
                        for e in range(ECH):
                            nc.tensor.matmul(
                                psall[m][e][:, :],
                                yt[:, j, m * MS : (m + 1) * MS],
                                w_sb[j][:, e * 512 : (e + 1) * 512],
                                start=(j == 0),
                                stop=(j == KT - 1),
                            )

                    def drain0(m):
                        for e in range(ECH):
                            os_sb = opool.tile(
                                [128, 512], dt.float32, name="os", tag="os"
                            )
                            nc.scalar.activation(
                                os_sb[:, :], psall[m][e][:, :], AF.Copy, bias=0.0
                            )
                            nc.vector.tensor_tensor(
                                out=os_sb[:, :],
                                in0=os_sb[:, :],
                                in1=bb_sb[:, e * 512 : (e + 1) * 512],
                                op=ALU.add,
                            )
                            nc.sync.dma_start(
                                out=out[
                                    (s + m) * MS : (s + m + 1) * MS,
                                    e * 512 : (e + 1) * 512,
                                ],
                                in_=os_sb[:, :],
                            )

                    for j in range(KT):
                        for m in range(nms):
                            mmj(m, j)
                    drain0(0)
                    drain0(1)
                    s += nms
                    continue

                for m in range(w // MS):
                    last_ms = s == T // MS - 1
                    pss = [
                        pspool.tile([128, 512], dt.float32, name="ps", tag="ps")
                        for _ in range(ECH)
                    ]

                    def drain(e):
                        # ACT drains the PSUM bank (frees it for the PE fast,
                        # and keeps the drain out of the DVE conv FIFO where
                        # it head-of-line blocks); DVE adds b_proj in place
                        # later - that only gates the out DMA, which has slack
                        os_sb = opool.tile([128, 512], dt.float32, name="os", tag="os")
                        nc.scalar.activation(
                            os_sb[:, :], pss[e][:, :], AF.Copy, bias=0.0
                        )
                        nc.vector.tensor_tensor(
                            out=os_sb[:, :],
                            in0=os_sb[:, :],
                            in1=bb_sb[:, e * 512 : (e + 1) * 512],
                            op=ALU.add,
                        )
                        nc.sync.dma_start(
                            out=out[s * MS : (s + 1) * MS, e * 512 : (e + 1) * 512],
                            in_=os_sb[:, :],
                        )

                    if last_ms:
                        # e-outer on the final matmul strip: each e-chunk's
                        # accumulation finishes early so its drain overlaps
                        # the remaining matmuls, shortening the kernel tail
                        for e in range(ECH):
                            for j in range(KT):
                                nc.tensor.matmul(
                                    pss[e][:, :],
                                    yt[:, j, m * MS : (m + 1) * MS],
                                    w_sb[j][:, e * 512 : (e + 1) * 512],
                                    start=(j == 0),
                                    stop=(j == KT - 1),
                                )
                            drain(e)
                    else:
                        # j-outer: 4 consecutive matmuls share the same
                        # stationary y tile; bf16 FWL weight loads pull
                        # ahead of the stream
                        for j in range(KT):
                            for e in range(ECH):
                                nc.tensor.matmul(
                                    pss[e][:, :],
                                    yt[:, j, m * MS : (m + 1) * MS],
                                    w_sb[j][:, e * 512 : (e + 1) * 512],
                                    start=(j == 0),
                                    stop=(j == KT - 1),
                                )
                        for e in range(ECH):
                            drain(e)
                    s += 1

    nc.compile()
    _BUILT["nc"] = nc
    return nc


def _shard_inputs(x, conv_w, conv_b, W_proj, b_proj):
    x = np.ascontiguousarray(x, dtype=np.float32).astype(BF16)
    # W^T packed as channel-tile pairs [8, 128, 2*2048]
    wt_np = np.ascontiguousarray(
        W_proj.T.astype(np.float32)
        .astype(BF16)
        .reshape(KT // 2, 2, 128, D)
        .transpose(0, 2, 1, 3)
    ).reshape(KT // 2, 128, 2 * D)
    cw_np = np.ascontiguousarray(
        conv_w.reshape(KT, 128, KW).transpose(1, 0, 2).reshape(128, KT * KW),
        dtype=np.float32,
    )
    cb_np = np.ascontiguousarray(conv_b.reshape(KT, 128).T, dtype=np.float32)
    bp_np = np.ascontiguousarray(b_proj.reshape(1, D), dtype=np.float32)

    starts = np.cumsum([0] + SW)[:-1]
    per_batch = S // T
    in_maps = []
    for c in range(NCORES):
        b = c // per_batch
        s0 = (c % per_batch) * T
        xp = np.zeros((T + 3, D), dtype=BF16)
        xp[3:] = x[b, s0 : s0 + T]
        if s0 > 0:
            xp[:3] = x[b, s0 - 3 : s0]
        xTc = xp.T  # [D, T+3]
        blocks = {w: [] for w in set(SW)}
        for st, w in zip(starts, SW):
            # [D, w+3] strip -> [16, 128, w+3] -> one [1, 128, 16*(w+3)] block
            strip = xTc[:, st : st + w + 3].reshape(KT, 128, w + 3)
            hv = np.ascontiguousarray(strip.transpose(1, 0, 2)).reshape(
                1, 128, KT * (w + 3)
            )
            blocks[w].append(hv)
        m = {
            "wt": wt_np,
            "cw": cw_np,
            "cb": cb_np,
            "bp": bp_np,
        }
        for w, bl in blocks.items():
            m[f"xs_{w}"] = np.concatenate(bl, axis=0)
        in_maps.append(m)
    return in_maps


def run_sharded(x, conv_w, conv_b, W_proj, b_proj, trace=False):
    """Run across the 8 cores; returns (full_out [B,S,D], BassKernelResults)."""
    from concourse.bass_utils import run_bass_kernel_spmd

    nc = _build_program()
    in_maps = _shard_inputs(x, conv_w, conv_b, W_proj, b_proj)
    try:
        res = run_bass_kernel_spmd(nc, in_maps, list(range(NCORES)), trace=trace)
    except Exception:
        # transient device wedges (NRT_EXEC_UNIT_UNRECOVERABLE) clear on retry
        res = run_bass_kernel_spmd(nc, in_maps, list(range(NCORES)), trace=trace)
    full = np.empty((B, S, D), dtype=np.float32)
    per_batch = S // T
    for c in range(NCORES):
        b = c // per_batch
        s0 = (c % per_batch) * T
        full[b, s0 : s0 + T] = res.results[c]["out"]
    return full, res


def kernel(x, conv_w, conv_b, W_proj, b_proj):
    full, _ = run_sharded(x, conv_w, conv_b, W_proj, b_proj, trace=False)
    return full


# revision 35
# speedup vs baseline: 1.2028x; 1.2028x over previous
"""Trainium2 Bass kernel for nn_Expert_13082470383822.

y = silu(depthwise_causal_conv1d(x, conv_w, K=4) + conv_b);  out = y @ W_proj.T + b_proj
x [4, 4096, 2048] fp32. Data-parallel over the 16384 (batch*seq) tokens across
8 NeuronCores (2048 tokens/core + 3-token halo).

Per-core: channels D on SBUF partitions. The whole data path runs in bf16 (host
casts x and W_proj; end-to-end error ~5e-3 vs the 2e-2 gate), which halves the
W stream (the fp32 baseline stalled the PE ~17us midway waiting on W tiles) and
the x strips. Conv strips are 256 tokens for the first two (short startup
chain), 512 after (amortizes the ~200-cycle fixed cost of each DVE/ACT op so
the conv no longer outpaces the PE's per-strip span). Per channel-tile: tap 0
on ACT (copy with per-partition scale), taps 1-3 as DVE scalar_tensor_tensor
chains (stt has no 2x uop - it runs 1x regardless of dtype), SiLU+conv_b on ACT
writing bf16 y tiles. Projection on the PE in bf16 (1 cycle/row, FWL weight
loads) accumulating fp32 in PSUM; b_proj added on the DVE drain. Output
streams out as [tokens, 2048] fp32 rows so the host gather is concatenation.
"""

import sys

if "/opt/trn_rl_repo" not in sys.path:
    sys.path.insert(0, "/opt/trn_rl_repo")

import numpy as np
import ml_dtypes

BF16 = ml_dtypes.bfloat16

B, S, D, KW = 4, 4096, 2048, 4
NCORES = 8
T = (B * S) // NCORES  # tokens per core = 2048
KT = D // 128  # 16 channel tiles
ECH = D // 512  # 4 e-chunks
MS = 128  # matmul strip width (tokens)
SW = [256] * 8  # conv strip widths (sum = T)
assert sum(SW) == T

_BUILT = {}


def _build_program():
    if "nc" in _BUILT:
        return _BUILT["nc"]

    import concourse.tile as tile
    from concourse import bacc, mybir

    dt = mybir.dt
    AF = mybir.ActivationFunctionType
    ALU = mybir.AluOpType

    nc = bacc.Bacc("TRN2", target_bir_lowering=False, debug=False)
    # pre-tiled x, one param per strip width, two j-halves per strip so the
    # conv starts on j0-7 while j8-15 stream in
    xs_p = {}
    for w in sorted(set(SW)):
        nblk = sum(1 for v in SW if v == w) * 2
        xs_p[w] = nc.declare_dram_parameter(
            f"xs_{w}", [nblk, 128, (KT // 2) * (w + 3)], dt.bfloat16, isOutput=False
        )
    wt = nc.declare_dram_parameter("wt", [D, D], dt.bfloat16, isOutput=False)
    cw = nc.declare_dram_parameter("cw", [128, KT * KW], dt.float32, isOutput=False)
    cb = nc.declare_dram_parameter("cb", [128, KT], dt.float32, isOutput=False)
    bp = nc.declare_dram_parameter("bp", [1, D], dt.float32, isOutput=False)
    out = nc.declare_dram_parameter("out", [T, D], dt.float32, isOutput=True)

    with tile.TileContext(nc) as tc:
        with (
            tc.tile_pool(name="consts", bufs=1) as cpool,
            tc.tile_pool(name="wpool", bufs=1) as wpool,
            tc.tile_pool(name="xpool", bufs=4) as xpool,
            tc.tile_pool(name="ypool", bufs=3) as ypool,
            tc.tile_pool(name="apool", bufs=4) as apool,
            tc.tile_pool(name="opool", bufs=8) as opool,
            tc.tile_pool(name="pspool", bufs=8, space="PSUM") as pspool,
        ):
            xq = []  # (strip) -> list of x tiles (1 block or 2 halves)
            blk_i = {w: 0 for w in xs_p}

            def load_strip(c):
                # strip 0 rides the fast HWDGE sync queue ahead of W so the
                # conv chain starts ~2us earlier; the rest stream on the
                # gpsimd SWDGE queue and never sit behind W
                eng = nc.sync if c < 1 else nc.gpsimd
                w = SW[c]
                xh = []
                for h in range(2):
                    xt_h = xpool.tile(
                        [128, KT // 2, w + 3], dt.bfloat16, name="xs", tag="xs"
                    )
                    sv = xs_p[w][blk_i[w] + h, :, :].rearrange(
                        "p (j t) -> p j t", j=KT // 2
                    )
                    eng.dma_start(out=xt_h[:, :, :], in_=sv)
                    xh.append(xt_h)
                blk_i[w] += 2
                xq.append(xh)

            dum = cpool.tile([1, 1], dt.float32, name="dum")
            nc.gpsimd.memset(dum[:, :], 0.0)
            # conv scale/bias ride the gpsimd queue head (tiny, land ~8.5us)
            # so the sync queue can lead with the x strips the conv chain
            # needs first, with W right behind
            cw_sb = cpool.tile([128, KT * KW], dt.float32, name="cw_sb")
            nc.gpsimd.dma_start(out=cw_sb[:, :], in_=cw[:, :])
            cb_sb = cpool.tile([128, KT], dt.float32, name="cb_sb")
            nc.gpsimd.dma_start(out=cb_sb[:, :], in_=cb[:, :])
            load_strip(0)
            load_strip(1)
            # warm the ACT function table before any real ACT work
            nc.scalar.activation(dum[:, :], dum[:, :], AF.Silu, bias=0.0)

            # W tiles stream on the SP HWDGE queue behind the strip-0 x,
            # one [128, 2048] tile per DMA - finer granularity delivers w_j
            # just-in-time for the strip-0 matmuls
            w_sb = []
            for j in range(KT):
                wj = wpool.tile([128, D], dt.bfloat16, name=f"w{j}")
                nc.sync.dma_start(out=wj[:, :], in_=wt[j * 128 : (j + 1) * 128, :])
                w_sb.append(wj)

            # bias broadcast rides the SP queue behind W: needed only by the
            # first drain, and it stays out of the x-strip FIFO
            bb_sb = cpool.tile([128, D], dt.float32, name="bb_sb")
            nc.sync.dma_start(out=bb_sb[:, :], in_=bp[:, :].broadcast_to([128, D]))

            s = 0  # running matmul-strip index
            for c in range(len(SW)):
                w = SW[c]
                if c + 2 < len(SW):
                    load_strip(c + 2)
                xh = xq[c]

                yt = ypool.tile([128, KT, w], dt.bfloat16, name="ys", tag="ys")

                for j in range(KT):
                    xs, jj = xh[j // 8], j % 8
                    acc = apool.tile([128, w], dt.bfloat16, name="acc", tag="acc")
                    # tap 0 on ACT: acc = w0 * x0
                    nc.scalar.activation(
                        acc[:, :],
                        xs[:, jj, 0:w],
                        AF.Copy,
                        bias=0.0,
                        scale=cw_sb[:, j * KW : j * KW + 1],
                    )  # tap 0
                    # taps 1-3 on DVE
                    for k in range(1, KW):
                        nc.vector.scalar_tensor_tensor(
                            acc[:, :],
                            xs[:, jj, k : k + w],
                            cw_sb[:, j * KW + k : j * KW + k + 1],
                            acc[:, :],
                            ALU.mult,
                            ALU.add,
                        )
                    # SiLU + conv bias on ACT, full strip width, bf16 out
                    nc.scalar.activation(
                        yt[:, j, :],
                        acc[:, :],
                        AF.Silu,
                        bias=cb_sb[:, j : j + 1],
                    )

                if c == 0:
                    # strip 0: open both matmul strips' PSUM groups at once
                    # (8 banks) and interleave them j-wise for j0-7 - the PE
                    # then wants a channel tile only every ~1.7us while the
                    # conv chain delivers every ~1.2us, so the stream starts
                    # dense instead of conv-paced. j8-15 finish m0 solo (the
                    # conv has banked enough lead by then) so m0's banks
                    # drain while m1 finishes, avoiding a strip-boundary
                    # bubble when strip 1 reuses them.
                    nms = w // MS
                    psall = [
                        [
                            pspool.tile([128, 512], dt.float32, name="ps", tag="ps")
                            for _ in range(ECH)
                        ]
                        for _ in range(nms)
                    ]

                    def mmj(m, j):
                        for e in range(ECH):
                            nc.tensor.matmul(
                                psall[m][e][:, :],
                                yt[:, j, m * MS : (m + 1) * MS],
                                w_sb[j][:, e * 512 : (e + 1) * 512],
                                start=(j == 0),
                                stop=(j == KT - 1),
                            )

                    def drain0(m):
                        for e in range(ECH):
                            os_sb = opool.tile(
                                [128, 512], dt.float32, name="os", tag="os"
                            )
                            nc.scalar.activation(
                                os_sb[:, :], psall[m][e][:, :], AF.Copy, bias=0.0
                            )
                            nc.vector.tensor_tensor(
                                out=os_sb[:, :],
                                in0=os_sb[:, :],
                                in1=bb_sb[:, e * 512 : (e + 1) * 512],
                                op=ALU.add,
                            )
                            nc.sync.dma_start(
                                out=out[
                                    (s + m) * MS : (s + m + 1) * MS,
                                    e * 512 : (e + 1) * 512,
                                ],
                                in_=os_sb[:, :],
                            )

                    for j in range(KT):
                        for m in range(nms):
                            mmj(m, j)
                    drain0(0)
                    drain0(1)
                    s += nms
                    continue

                for m in range(w // MS):
                    last_ms = s == T // MS - 1
                    pss = [
                        pspool.tile([128, 512], dt.float32, name="ps", tag="ps")
                        for _ in range(ECH)
                    ]

                    def drain(e):
                        # ACT drains the PSUM bank (frees it for the PE fast,
                        # and keeps the drain out of the DVE conv FIFO where
                        # it head-of-line blocks); DVE adds b_proj in place
                        # later - that only gates the out DMA, which has slack
                        os_sb = opool.tile([128, 512], dt.float32, name="os", tag="os")
                        nc.scalar.activation(
                            os_sb[:, :], pss[e][:, :], AF.Copy, bias=0.0
                        )
                        nc.vector.tensor_tensor(
                            out=os_sb[:, :],
                            in0=os_sb[:, :],
                            in1=bb_sb[:, e * 512 : (e + 1) * 512],
                            op=ALU.add,
                        )
                        nc.sync.dma_start(
                            out=out[s * MS : (s + 1) * MS, e * 512 : (e + 1) * 512],
                            in_=os_sb[:, :],
                        )

                    if last_ms:
                        # e-outer on the final matmul strip: each e-chunk's
                        # accumulation finishes early so its drain overlaps
                        # the remaining matmuls, shortening the kernel tail
                        for e in range(ECH):
                            for j in range(KT):
                                nc.tensor.matmul(
                                    pss[e][:, :],
                                    yt[:, j, m * MS : (m + 1) * MS],
                                    w_sb[j][:, e * 512 : (e + 1) * 512],
                                    start=(j == 0),
                                    stop=(j == KT - 1),
                                )
                            drain(e)
                    else:
                        # j-outer: 4 consecutive matmuls share the same
                        # stationary y tile; bf16 FWL weight loads pull
                        # ahead of the stream
                        for j in range(KT):
                            for e in range(ECH):
                                nc.tensor.matmul(
                                    pss[e][:, :],
                                    yt[:, j, m * MS : (m + 1) * MS],
                                    w_sb[j][:, e * 512 : (e + 1) * 512],
                                    start=(j == 0),
                                    stop=(j == KT - 1),
                                )
                        for e in range(ECH):
                            drain(e)
                    s += 1

    nc.compile()
    _BUILT["nc"] = nc
    return nc


def _shard_inputs(x, conv_w, conv_b, W_proj, b_proj):
    x = np.ascontiguousarray(x, dtype=np.float32).astype(BF16)
    wt_np = np.ascontiguousarray(W_proj.T, dtype=np.float32).astype(BF16)
    cw_np = np.ascontiguousarray(
        conv_w.reshape(KT, 128, KW).transpose(1, 0, 2).reshape(128, KT * KW),
        dtype=np.float32,
    )
    cb_np = np.ascontiguousarray(conv_b.reshape(KT, 128).T, dtype=np.float32)
    bp_np = np.ascontiguousarray(b_proj.reshape(1, D), dtype=np.float32)

    starts = np.cumsum([0] + SW)[:-1]
    per_batch = S // T
    in_maps = []
    for c in range(NCORES):
        b = c // per_batch
        s0 = (c % per_batch) * T
        xp = np.zeros((T + 3, D), dtype=BF16)
        xp[3:] = x[b, s0 : s0 + T]
        if s0 > 0:
            xp[:3] = x[b, s0 - 3 : s0]
        xTc = xp.T  # [D, T+3]
        blocks = {w: [] for w in set(SW)}
        for st, w in zip(starts, SW):
            # [D, w+3] strip -> [16, 128, w+3] -> 2 j-halves [2, 128, 8*(w+3)]
            strip = xTc[:, st : st + w + 3].reshape(KT, 128, w + 3)
            hv = np.ascontiguousarray(
                strip.reshape(2, KT // 2, 128, w + 3).transpose(0, 2, 1, 3)
            ).reshape(2, 128, (KT // 2) * (w + 3))
            blocks[w].append(hv)
        m = {
            "wt": wt_np,
            "cw": cw_np,
            "cb": cb_np,
            "bp": bp_np,
        }
        for w, bl in blocks.items():
            m[f"xs_{w}"] = np.concatenate(bl, axis=0)
        in_maps.append(m)
    return in_maps


def run_sharded(x, conv_w, conv_b, W_proj, b_proj, trace=False):
    """Run across the 8 cores; returns (full_out [B,S,D], BassKernelResults)."""
    from concourse.bass_utils import run_bass_kernel_spmd

    nc = _build_program()
    in_maps = _shard_inputs(x, conv_w, conv_b, W_proj, b_proj)
    try:
        res = run_bass_kernel_spmd(nc, in_maps, list(range(NCORES)), trace=trace)
    except Exception:
        # transient device wedges (NRT_EXEC_UNIT_UNRECOVERABLE) clear on retry
        res = run_bass_kernel_spmd(nc, in_maps, list(range(NCORES)), trace=trace)
    full = np.empty((B, S, D), dtype=np.float32)
    per_batch = S // T
    for c in range(NCORES):
        b = c // per_batch
        s0 = (c % per_batch) * T
        full[b, s0 : s0 + T] = res.results[c]["out"]
    return full, res


def kernel(x, conv_w, conv_b, W_proj, b_proj):
    full, _ = run_sharded(x, conv_w, conv_b, W_proj, b_proj, trace=False)
    return full
